# revision 1
# baseline (speedup 1.0000x reference)
"""Trainium2 Bass kernel for OctahedralCavityProcessor (bf16 + SBUF-resident x).

Sharding: data-parallel over batch (B=8 -> 8 cores, zero collectives).
Per core (one batch element), three phases:
  A: cavity pooling  sums[k,c] = sum_p x[c,p] * mask[p,k]
     - x arrives bf16 (host-cast); most of it is loaded once into a big
       SBUF-resident tile and PE-transposed in 128-pt chunks; the tail is
       loaded via DMA-transposing loads. One long PSUM accumulation.
  B: per-cavity MLP + 14-token attention, all bf16 weights (tiny).
  C: out[c,p] = x[c,p] + att[c, nearest[p]]
     - gather via matmul with bf16 onehot; the +x is a second matmul with
       a bf16 identity into the same PSUM; PSUM->SBUF copies alternate
       Activation/DVE; resident columns are overwritten in place and
       DMAed out in 16KB/partition blocks; out is bf16 (host upcasts).

Numerics: tolerance is 2e-2 relative (~0.108 abs); bf16 rounding of x
contributes <=0.031 abs, everything else is far smaller.
"""

import numpy as np
import ml_dtypes

import concourse.bass as bass
import concourse.tile as tile
from concourse import mybir
from concourse.bass_utils import run_bass_kernel_spmd
from concourse.vector_clock import ScopedClock, VectorClock
from contextlib import ExitStack

F32 = mybir.dt.float32
BF16 = mybir.dt.bfloat16
NPBF16 = ml_dtypes.bfloat16

B, C, P, K, H = 8, 128, 100000, 14, 8
C2 = 2 * C
Dh = C // H
RADIUS = np.float32(0.5)

CHA = 128                      # transpose chunk (points)
PP = ((P + CHA - 1) // CHA) * CHA   # 100096, x padded to this
NA = PP // CHA                 # 782 chunks, all full
XB = 2048                      # x load block (columns)
R_RES = 81920                  # resident columns (40 XB blocks)
NRB = R_RES // XB              # 40 resident load blocks
RCH = R_RES // CHA             # 640 resident chunks
MG = 64                        # mask chunks per streamed mask tile
CHC = 512                      # phase C psum chunk
OB = 8192                      # phase C resident out-store block

# consts packed layout (columns in constC [C, *], bf16)
OFF_W1 = 0                     # [C, K*C2]  w1t
OFF_W2 = OFF_W1 + K * C2       # [C, K*2*C] w2t
OFF_WQ = OFF_W2 + K * 2 * C
OFF_WK = OFF_WQ + C
OFF_WV = OFF_WK + C
OFF_B1 = OFF_WV + C            # [C, 2K] b1 broadcast
OFF_B2 = OFF_B1 + 2 * K        # [C, K]
OFF_ID = OFF_B2 + K            # [C, C] identity (bf16)
NCONSTC = OFF_ID + C


def _legalize_bir_waits(bir_json: bytes) -> bytes:
    """walrus here accepts at most ONE sync-wait command per instruction.
    Tile's scheduler may attach several.  Hoist the extras onto NoOp
    instructions inserted immediately before, on the same engine (the
    engine executes serially, so waiting one-at-a-time is equivalent)."""
    import json as _json

    d = _json.loads(bir_json)
    changed = False
    for fn in d.get("functions", []):
        for blk in fn.get("blocks", []):
            insts = blk.get("instructions", [])
            out = []
            for ins in insts:
                waits = (ins.get("sync_info") or {}).get("on_wait", [])
                if len(waits) > 1:
                    changed = True
                    for i, w in enumerate(waits[:-1]):
                        out.append({
                            "debug": ins.get("debug", 0),
                            "engine": ins["engine"],
                            "ins": [],
                            "name": f"{ins['name']}-wsplit{i}",
                            "opcode": "NoOp",
                            "outs": [],
                            "sync_info": {"on_update": [], "on_wait": [w]},
                            "text_hint": "wait_split",
                        })
                    ins["sync_info"]["on_wait"] = [waits[-1]]
                out.append(ins)
            blk["instructions"] = out
    if not changed:
        return bir_json
    return _json.dumps(d).encode()


def _install_wait_legalizer():
    import concourse.bass2jax as _b2j

    orig = _b2j.compile_bir_kernel
    if getattr(orig, "_wait_legalized", False):
        return

    def patched(bir_json, tmpdir, neff_name="file.neff"):
        return orig(_legalize_bir_waits(bir_json), tmpdir, neff_name=neff_name)

    patched._wait_legalized = True
    _b2j.compile_bir_kernel = patched


_install_wait_legalizer()


class SplitDrainTileContext(tile.TileContext):
    """The walrus build here only accepts ONE sync-wait command per
    instruction; stock TileContext puts every live sem wait on the tail
    Drain.  Split them across nop instructions instead."""

    def _drain_and_barrier(self, tick_clock, wait_clock):
        gc = tick_clock.global_clock
        n = len(gc)
        for i in range(n):
            if gc[i] <= 0:
                continue
            vec = [gc[j] if j == i else 0 for j in range(n)]
            nop = self.nc.sync.nop(nofuse=True, hint="tail_drain_split")
            wait_clock.add_sem_waits(nop.ins, ScopedClock({None: VectorClock(vec)}))
        self.nc.sync.drain()
        self.nc.all_engine_barrier()
        assert self.sems is not None
        popped = self.nc._tile_sem_poison_stack.pop()
        assert popped is self._sem_poison
        self.nc.clear_and_free_semaphores(list(self.sems.allocated().values()))
        self.nc.all_engine_barrier()


def build_program(reps=1):
    nc = bass.Bass()

    x_d = nc.dram_tensor("x", [C, PP], BF16, kind="ExternalInput")
    maskA_d = nc.dram_tensor("maskA", [CHA, NA * K], BF16, kind="ExternalInput")
    onehot_d = nc.dram_tensor("onehot", [K, P], BF16, kind="ExternalInput")
    ident_d = nc.dram_tensor("ident", [C, C], BF16, kind="ExternalInput")
    constC_d = nc.dram_tensor("constC", [C, NCONSTC], BF16, kind="ExternalInput")
    constD_d = nc.dram_tensor("constD", [Dh, H * C + 2 * H], BF16,
                              kind="ExternalInput")
    vb_d = nc.dram_tensor("vb", [K, C], BF16, kind="ExternalInput")
    ob_d = nc.dram_tensor("ob", [C, 1], F32, kind="ExternalInput")
    inv_d = nc.dram_tensor("inv", [K, 1], F32, kind="ExternalInput")
    out_d = nc.dram_tensor("out", [C, P], BF16, kind="ExternalOutput")

    Id = mybir.ActivationFunctionType.Identity

    with SplitDrainTileContext(nc) as tc:
      for _rep in range(reps):
        with ExitStack() as octx:
            cpool = octx.enter_context(tc.tile_pool(name="consts", bufs=1))

            ident_s = cpool.tile([C, C], BF16, tag="ident")
            nc.sync.dma_start(ident_s[:], ident_d[:])
            constC = cpool.tile([C, NCONSTC], BF16, tag="constC")
            nc.scalar.dma_start(constC[:], constC_d[:])
            constD = cpool.tile([Dh, H * C + 2 * H], BF16, tag="constD")
            nc.scalar.dma_start(constD[:], constD_d[:])
            vb_s = cpool.tile([K, C], BF16, tag="vb")
            nc.scalar.dma_start(vb_s[:], vb_d[:])
            ob_s = cpool.tile([C, 1], F32, tag="ob")
            nc.scalar.dma_start(ob_s[:], ob_d[:])
            inv_s = cpool.tile([K, 1], F32, tag="inv")
            nc.scalar.dma_start(inv_s[:], inv_d[:])

            identC = ident_s[:]
            identK = ident_s[:K, :K]

            # resident x (bf16): loaded once, pooled from, overwritten by
            # phase C, stored to out.
            xres = cpool.tile([C, R_RES], BF16, tag="xres")

            # ---------------- phase A: cavity pooling ----------------
            abctx = ExitStack()
            sums_pool = abctx.enter_context(
                tc.tile_pool(name="sums_ps", bufs=1, space="PSUM"))
            sums_ps = sums_pool.tile([K, C], F32, tag="sums")
            with ExitStack() as actx:
                m_pool = actx.enter_context(tc.tile_pool(name="mA", bufs=2))
                xt_pool = actx.enter_context(tc.tile_pool(name="xtA", bufs=3))
                xs_pool = actx.enter_context(tc.tile_pool(name="xtS", bufs=3))
                tp_pool = actx.enter_context(
                    tc.tile_pool(name="tpA", bufs=6, space="PSUM"))

                # streamed-tail masks: one small tile, loaded once
                SCH = NA - RCH
                msS = m_pool.tile([CHA, SCH * K], BF16, tag="msS")
                nc.gpsimd.dma_start(msS[:], maskA_d[:, RCH * K:NA * K])

                m_state = {"g": -1, "t": None}

                def mask_chunk(c):
                    """SBUF slice [CHA, K] for chunk c.  Resident chunks
                    stream MG chunks per tile in order (Pool-engine SWDGE
                    so waits don't block other sequencers); streamed-tail
                    chunks read the one-shot msS tile."""
                    if c >= RCH:
                        j = c - RCH
                        return msS[:, j * K:(j + 1) * K]
                    g, j = divmod(c, MG)
                    if m_state["g"] != g:
                        mw = min(MG, RCH - g * MG) * K
                        m_t = m_pool.tile([CHA, MG * K], BF16, tag="m")
                        nc.gpsimd.dma_start(
                            m_t[:, :mw], maskA_d[:, g * MG * K:g * MG * K + mw])
                        m_state["g"] = g
                        m_state["t"] = m_t
                    return m_state["t"][:, j * K:(j + 1) * K]

                acc_n = [0]

                def accum(lhsT, rhs):
                    nc.tensor.matmul(sums_ps[:], lhsT=lhsT, rhs=rhs,
                                     start=(acc_n[0] == 0),
                                     stop=(acc_n[0] == NA - 1))
                    acc_n[0] += 1

                # streamed-tail blocks (DMA-transposing loads straight to
                # [p, c]) interleaved into the resident loop so the DMAs
                # spread across phase A instead of serializing at the end
                NSB = (PP - R_RES + XB - 1) // XB
                NG4 = RCH // 4

                xs_tiles = {}

                def load_stream_block(sb):
                    if sb >= NSB:
                        return
                    b0 = R_RES + sb * XB
                    bw = min(XB, PP - b0)
                    nch = bw // CHA
                    xs_t = xs_pool.tile([CHA, XB // CHA, C], BF16, tag="xss")
                    nc.sync.dma_start_transpose(
                        xs_t[:, :nch, :], x_d[:, b0:b0 + bw])
                    xs_tiles[sb] = (xs_t, nch)

                def do_stream_block(sb):
                    load_stream_block(sb + 1)
                    xs_t, nch = xs_tiles.pop(sb)
                    for j in range(nch):
                        c = RCH + sb * (XB // CHA) + j
                        accum(mask_chunk(c), xs_t[:, j, :])

                # resident region: plain loads + PE transposes, 8 chunks
                # (one full 2KB bf16 PSUM bank) per group; PSUM->SBUF
                # copies alternate DVE/Activation
                GC = 8
                NG8 = RCH // GC
                load_stream_block(0)
                sb_next = 0
                for g8 in range(NG8):
                    cs = [g8 * GC + i for i in range(GC)]
                    if g8 % (XB // (GC * CHA)) == 0:
                        b0 = g8 * GC * CHA
                        nc.sync.dma_start(xres[:, b0:b0 + XB],
                                          x_d[:, b0:b0 + XB])
                    tp_t = tp_pool.tile([CHA, GC * C], BF16, tag="tp")
                    for i, c in enumerate(cs):
                        nc.tensor.matmul(
                            tp_t[:, i * C:(i + 1) * C],
                            lhsT=xres[:, c * CHA:(c + 1) * CHA],
                            rhs=identC,
                            is_transpose=True,
                            start=(i == 0), stop=(i == GC - 1),
                        )
                    xt_s = xt_pool.tile([CHA, GC * C], BF16, tag="xts")
                    if g8 % 2 == 0:
                        nc.vector.tensor_copy(xt_s[:], tp_t[:])
                    else:
                        nc.scalar.activation(xt_s[:], tp_t[:], Id)
                    for i, c in enumerate(cs):
                        accum(mask_chunk(c), xt_s[:, i * C:(i + 1) * C])
                    if (g8 + 1) % max(1, NG8 // NSB) == 0 and sb_next < NSB:
                        do_stream_block(sb_next)
                        sb_next += 1
                while sb_next < NSB:
                    do_stream_block(sb_next)
                    sb_next += 1

            # ---------------- phase B: MLP + attention ----------------
            with ExitStack() as bctx:
                bp = bctx.enter_context(tc.tile_pool(name="bp", bufs=2))
                sp = bctx.enter_context(
                    tc.tile_pool(name="sp_ps", bufs=4, space="PSUM"))

                # feat (mean) in bf16: sums * inv[k], then transpose to [C,K]
                f_s = bp.tile([K, C], BF16, tag="f_s")
                nc.vector.tensor_scalar_mul(f_s[:], sums_ps[:], inv_s[:])
                tpf = sp.tile([C, K], BF16, tag="sps")
                nc.tensor.matmul(tpf[:], lhsT=f_s[:], rhs=identK,
                                 is_transpose=True, start=True, stop=True)
                featT = bp.tile([C, K], BF16, tag="featT")
                nc.vector.tensor_copy(featT[:], tpf[:])

                # h = relu(w1 @ feat + b1): 28 matmuls into one psum group
                ph = sp.tile([C, 2 * K], F32, tag="sps")
                for k in range(K):
                    for half in range(2):
                        w1c = OFF_W1 + k * C2 + half * C
                        nc.tensor.matmul(
                            ph[:, 2 * k + half:2 * k + half + 1],
                            lhsT=constC[:, w1c:w1c + C],
                            rhs=featT[:, k:k + 1],
                            start=(k == 0 and half == 0),
                            stop=(k == K - 1 and half == 1),
                        )
                hb = bp.tile([C, 2 * K], BF16, tag="hb")
                nc.vector.tensor_tensor(
                    hb[:], ph[:], constC[:, OFF_B1:OFF_B1 + 2 * K],
                    op=mybir.AluOpType.add)
                h_s = bp.tile([C, 2 * K], BF16, tag="h")
                nc.vector.tensor_scalar_max(h_s[:], hb[:], 0.0)

                # proc = tanh(w2 @ h + b2): paired accumulation per cavity
                pp = sp.tile([C, K], F32, tag="sps")
                for k in range(K):
                    for half in range(2):
                        w2c = OFF_W2 + (k * 2 + half) * C
                        nc.tensor.matmul(
                            pp[:, k:k + 1],
                            lhsT=constC[:, w2c:w2c + C],
                            rhs=h_s[:, 2 * k + half:2 * k + half + 1],
                            start=(half == 0), stop=(half == 1),
                        )
                pb = bp.tile([C, K], BF16, tag="pb")
                nc.vector.tensor_tensor(
                    pb[:], pp[:], constC[:, OFF_B2:OFF_B2 + K],
                    op=mybir.AluOpType.add)
                procT = bp.tile([C, K], BF16, tag="procT")
                nc.scalar.activation(procT[:], pb[:],
                                     mybir.ActivationFunctionType.Tanh)

                # ---- attention over K=14 cavities (head-blocked) ----
                wo_s = constD[:, 0:H * C]
                qb_s = constD[:, H * C:H * C + H]
                kb_s = constD[:, H * C + H:H * C + 2 * H]

                pq = sp.tile([Dh, H * K], F32, tag="sps")
                for h in range(H):
                    nc.tensor.matmul(pq[:, h * K:(h + 1) * K],
                                     lhsT=constC[:, OFF_WQ + h * Dh:
                                                 OFF_WQ + (h + 1) * Dh],
                                     rhs=procT[:],
                                     start=(h == 0), stop=(h == H - 1))
                qh_s = bp.tile([Dh, H * K], BF16, tag="qT")
                for h in range(H):
                    nc.scalar.activation(qh_s[:, h * K:(h + 1) * K],
                                         pq[:, h * K:(h + 1) * K], Id,
                                         bias=qb_s[:, h:h + 1])

                pk = sp.tile([Dh, H * K], F32, tag="sps")
                for h in range(H):
                    nc.tensor.matmul(pk[:, h * K:(h + 1) * K],
                                     lhsT=constC[:, OFF_WK + h * Dh:
                                                 OFF_WK + (h + 1) * Dh],
                                     rhs=procT[:],
                                     start=(h == 0), stop=(h == H - 1))
                kh_s = bp.tile([Dh, H * K], BF16, tag="kT")
                for h in range(H):
                    nc.scalar.activation(kh_s[:, h * K:(h + 1) * K],
                                         pk[:, h * K:(h + 1) * K], Id,
                                         bias=kb_s[:, h:h + 1])

                pv = sp.tile([K, C], F32, tag="sps")
                nc.tensor.matmul(pv[:], lhsT=procT[:],
                                 rhs=constC[:, OFF_WV:OFF_WV + C])
                v_s = bp.tile([K, C], BF16, tag="v")
                nc.vector.tensor_add(v_s[:], pv[:], vb_s[:])

                psc = sp.tile([K, H * K], F32, tag="sps")
                for h in range(H):
                    nc.tensor.matmul(
                        psc[:, h * K:(h + 1) * K],
                        lhsT=qh_s[:, h * K:(h + 1) * K],
                        rhs=kh_s[:, h * K:(h + 1) * K],
                        start=(h == 0), stop=(h == H - 1),
                    )
                negmax = bp.tile([K, H], F32, tag="negmax")
                nc.vector.tensor_reduce(
                    out=negmax[:],
                    in_=psc[:].rearrange("p (h j) -> p h j", j=K),
                    op=mybir.AluOpType.max,
                    axis=mybir.AxisListType.X,
                    negate=True,
                )
                esc = bp.tile([K, H * K], BF16, tag="esc")
                for h in range(H):
                    nc.scalar.activation(
                        esc[:, h * K:(h + 1) * K], psc[:, h * K:(h + 1) * K],
                        mybir.ActivationFunctionType.Exp,
                        bias=negmax[:, h:h + 1],
                    )
                ssum = bp.tile([K, H], F32, tag="ssum")
                nc.vector.tensor_reduce(
                    out=ssum[:],
                    in_=esc[:].rearrange("p (h j) -> p h j", j=K),
                    op=mybir.AluOpType.add,
                    axis=mybir.AxisListType.X,
                )
                rinv = bp.tile([K, H], F32, tag="rinv")
                nc.vector.reciprocal(rinv[:], ssum[:])
                for h in range(H):
                    nc.vector.tensor_scalar_mul(
                        esc[:, h * K:(h + 1) * K], esc[:, h * K:(h + 1) * K],
                        rinv[:, h:h + 1],
                    )

                pat = sp.tile([K, H * K], BF16, tag="sps")
                for h in range(H):
                    nc.tensor.matmul(
                        pat[:, h * K:(h + 1) * K],
                        lhsT=esc[:, h * K:(h + 1) * K],
                        rhs=identK,
                        is_transpose=True,
                        start=(h == 0), stop=(h == H - 1),
                    )
                at_s = bp.tile([K, H * K], BF16, tag="at")
                nc.vector.tensor_copy(at_s[:], pat[:])

                po = sp.tile([Dh, H * K], F32, tag="sps")
                for h in range(H):
                    nc.tensor.matmul(
                        po[:, h * K:(h + 1) * K],
                        lhsT=v_s[:, h * Dh:(h + 1) * Dh],
                        rhs=at_s[:, h * K:(h + 1) * K],
                        start=(h == 0), stop=(h == H - 1),
                    )
                o_s = bp.tile([Dh, H * K], BF16, tag="o")
                nc.vector.tensor_copy(o_s[:], po[:])

                patt = sp.tile([C, K], F32, tag="sps")
                for h in range(H):
                    nc.tensor.matmul(patt[:],
                                     lhsT=wo_s[:, h * C:(h + 1) * C],
                                     rhs=o_s[:, h * K:(h + 1) * K],
                                     start=(h == 0), stop=(h == H - 1))
                attT_s = bp.tile([C, K], BF16, tag="attT")
                nc.scalar.activation(attT_s[:], patt[:], Id, bias=ob_s[:])

                pak = sp.tile([K, C], BF16, tag="sps")
                nc.tensor.matmul(pak[:], lhsT=attT_s[:], rhs=identC,
                                 is_transpose=True, start=True, stop=True)
                ak_s = cpool.tile([K, C], BF16, tag="ak")
                nc.vector.tensor_copy(ak_s[:], pak[:])
            abctx.close()

            # ---------------- phase C: gather-add ----------------
            with ExitStack() as cctx:
                oh_pool = cctx.enter_context(tc.tile_pool(name="oh", bufs=4))
                xc_pool = cctx.enter_context(tc.tile_pool(name="xc", bufs=3))
                pc_pool = cctx.enter_context(
                    tc.tile_pool(name="pc", bufs=8, space="PSUM"))

                oh_t = None

                def oh_chunk(p0, w):
                    """SBUF slice [K, w] of onehot for points p0:p0+w."""
                    nonlocal oh_t
                    g, j = divmod(p0, XB)
                    if j == 0:
                        gw = min(XB, P - g * XB)
                        oh_t = oh_pool.tile([K, XB], BF16, tag="oh")
                        nc.scalar.dma_start(oh_t[:, :gw],
                                            onehot_d[:, g * XB:g * XB + gw])
                    return oh_t[:, j:j + w]

                pend = []

                def flush_pair():
                    """Emit buffered chunks as MM1,MM1,MM2,MM2,copy,copy so
                    the PE reloads each stationary once per pair."""
                    for pc_t, dst, w, _ in pend:
                        nc.tensor.matmul(pc_t[:, :w], lhsT=ak_s[:],
                                         rhs=oh_chunk(*_),
                                         start=True, stop=False)
                    for pc_t, dst, w, _ in pend:
                        nc.tensor.matmul(pc_t[:, :w], lhsT=identC,
                                         rhs=dst, start=False, stop=True)
                    for i, (pc_t, dst, w, _) in enumerate(pend):
                        if i % 2 == 0:
                            nc.scalar.activation(dst, pc_t[:, :w], Id)
                        else:
                            nc.vector.tensor_copy(dst, pc_t[:, :w])
                    pend.clear()

                def chunk_ops(dst, p0, w, parity):
                    """psum = att@onehot + x (identity matmul); copy back."""
                    pc_t = pc_pool.tile([C, CHC], F32, tag="pc")
                    pend.append((pc_t, dst, w, (p0, w)))
                    if len(pend) == 4:
                        flush_pair()

                # streamed-region blocks are interleaved between resident
                # OB blocks; their x loads are issued one block early
                NSC = (P - R_RES + XB - 1) // XB
                xc_tiles = {}

                def load_stream_c(i):
                    if i >= NSC:
                        return
                    b0 = R_RES + i * XB
                    bw = min(XB, P - b0)
                    xc_t = xc_pool.tile([C, XB], BF16, tag="xc")
                    nc.sync.dma_start(xc_t[:, :bw], x_d[:, b0:b0 + bw])
                    xc_tiles[i] = (xc_t, b0, bw)

                nchunk = 0

                def do_stream_c(i):
                    nonlocal nchunk
                    load_stream_c(i + 1)
                    xc_t, b0, bw = xc_tiles.pop(i)
                    for j in range(0, bw, CHC):
                        w = min(CHC, bw - j)
                        chunk_ops(xc_t[:, j:j + w], b0 + j, w, nchunk)
                        nchunk += 1
                    nc.sync.dma_start(out_d[:, b0:b0 + bw], xc_t[:, :bw])

                load_stream_c(0)
                for rb in range(R_RES // OB):
                    ob0 = rb * OB
                    for p0 in range(ob0, ob0 + OB, CHC):
                        chunk_ops(xres[:, p0:p0 + CHC], p0, CHC, nchunk)
                        nchunk += 1
                    nc.sync.dma_start(out_d[:, ob0:ob0 + OB],
                                      xres[:, ob0:ob0 + OB])
                    if rb < NSC:
                        do_stream_c(rb)

    return nc


def prep_host(points, cavities, w1, b1, w2, b2, in_w, in_b, out_w, out_b):
    """Geometry + weight preprocessing (pure numpy, no x dependence)."""
    points = np.asarray(points, np.float32)
    cavities = np.asarray(cavities, np.float32)
    d = np.sqrt(
        ((points[None, :, :] - cavities[:, None, :]) ** 2).sum(-1, dtype=np.float32)
    ).astype(np.float32)                                   # [K, P]
    mask = (d < RADIUS).astype(np.float32)                 # [K, P]
    counts = mask.sum(axis=1, dtype=np.float32)            # [K]
    inv = np.where(counts > 0, 1.0 / np.maximum(counts, 1.0), 0.0).astype(np.float32)

    maskA = np.zeros((NA * CHA, K), np.float32)
    maskA[:P] = mask.T
    # [CHA, NA*K]: maskA[p, c*K+k] = mask[k, c*128+p]
    maskA = maskA.reshape(NA, CHA, K).transpose(1, 0, 2).reshape(CHA, NA * K)

    nearest = np.argmin(d, axis=0)                         # [P]
    onehot = np.zeros((K, P), np.float32)
    onehot[nearest, np.arange(P)] = 1.0

    w1 = np.asarray(w1, np.float32)
    w2 = np.asarray(w2, np.float32)
    in_w = np.asarray(in_w, np.float32)
    in_b = np.asarray(in_b, np.float32)
    out_w = np.asarray(out_w, np.float32)
    scale = np.float32(1.0 / np.sqrt(Dh))

    constC = np.zeros((C, NCONSTC), np.float32)
    constC[:, OFF_W1:OFF_W1 + K * C2] = (
        w1.transpose(0, 2, 1).reshape(K * C2 * C)          # w1t[k][c, d]
        .reshape(K, C, C2).transpose(1, 0, 2).reshape(C, K * C2))
    constC[:, OFF_W2:OFF_W2 + K * 2 * C] = (
        w2.transpose(0, 2, 1).reshape(K, C2, C)            # w2t[k][d, e]
        .reshape(K * 2, C, C).transpose(1, 0, 2).reshape(C, K * 2 * C))
    constC[:, OFF_WQ:OFF_WQ + C] = in_w[0:C].T * scale
    constC[:, OFF_WK:OFF_WK + C] = in_w[C:2 * C].T
    constC[:, OFF_WV:OFF_WV + C] = in_w[2 * C:3 * C].T
    constC[:, OFF_B1:OFF_B1 + 2 * K] = (
        np.asarray(b1, np.float32).reshape(K, 2, C).transpose(2, 0, 1)
        .reshape(C, 2 * K))
    constC[:, OFF_B2:OFF_B2 + K] = np.asarray(b2, np.float32).T
    constC[:, OFF_ID:OFF_ID + C] = np.eye(C, dtype=np.float32)

    constD = np.zeros((Dh, H * C + 2 * H), np.float32)
    constD[:, 0:H * C] = (out_w.reshape(C, H, Dh)
                          .transpose(2, 1, 0).reshape(Dh, H * C))
    constD[:, H * C:H * C + H] = (in_b[0:C] * scale).reshape(H, Dh).T
    constD[:, H * C + H:H * C + 2 * H] = in_b[C:2 * C].reshape(H, Dh).T

    fp = {
        "ident": np.eye(C, dtype=np.float32).astype(NPBF16),
        "maskA": maskA.astype(NPBF16),
        "onehot": onehot.astype(NPBF16),
        "constC": constC.astype(NPBF16),
        "constD": constD.astype(NPBF16),
        "vb": np.tile(in_b[2 * C:3 * C], (K, 1)).astype(NPBF16),
        "ob": np.ascontiguousarray(np.asarray(out_b, np.float32).reshape(C, 1)),
        "inv": np.ascontiguousarray(inv.reshape(K, 1)),
    }
    return fp


_PROGRAM = None


def kernel(x, points, cavities, w1, b1, w2, b2, in_w, in_b, out_w, out_b):
    global _PROGRAM
    x = np.asarray(x, np.float32)
    fp = prep_host(points, cavities, w1, b1, w2, b2, in_w, in_b, out_w, out_b)
    if _PROGRAM is None:
        _PROGRAM = build_program()
    nc = _PROGRAM
    xb = np.zeros((B, C, PP), NPBF16)
    xb[:, :, :P] = x.astype(NPBF16)
    in_maps = [dict(fp, x=xb[b]) for b in range(B)]
    res = run_bass_kernel_spmd(nc, in_maps, list(range(B)))
    out = np.stack(
        [np.asarray(res.results[b]["out"], np.float32) for b in range(B)],
        axis=0)
    return out

